# revision 6
# baseline (speedup 1.0000x reference)
"""Trainium2 Bass kernel for nn_Attention_27358941675773.

Reference computation (per batch b):
    q = x @ Q              [N, H]
    k = x @ K              [N, H]
    V = V_down @ V_up      [L, L]
    v = x @ V              [N, L]
    S = q @ k.T / 256      [N, N]
    out = softmax(S) @ v   [N, L]

Sharding: pure data-parallel over batch B=8 across the 8 NeuronCores
(one batch element per core); small params replicated. No collectives.

Per-core kernel strategy (N=4096, L=256, H=128):
  - All projection inputs cast to fp16 on-chip; matmuls run at full PE
    rate (1 cyc/row). qT [H,N] and kT [H,N] are computed directly in
    transposed layout so that scores can be built as S_T[m, n] (keys on
    partitions) without any transposes.
  - exp(S_T/256) is computed on the Scalar engine straight out of PSUM,
    written as bf16 (scores can reach ~±70; exp stays in fp32/bf16
    range, so no max-subtraction pass is needed).
  - softmax denominator rowsum[n] = sum_m exp(S_T[m,n]) via a ones-vector
    matmul accumulated in PSUM (reduction over the partition axis).
  - numerator out^T[l, n] = sum_m v[m, l] * expS_T[m, n] via matmul with
    v kept in its natural [m, l] layout - again no transposes.
  - final: out^T tiles are PE-transposed back to [n, l], scaled by
    1/rowsum (per-partition scalar on the Scalar engine) and DMA'd out.
  - The PV matmul of block k-1 is software-pipelined against the
    QK/exp of block k so the Scalar engine's exp stream stays hidden.
"""

import os
import sys

import numpy as np

for _p in ("/opt/trn_rl_repo",):
    if _p not in sys.path and os.path.isdir(_p):
        sys.path.insert(0, _p)

B, N, L, H = 8, 4096, 256, 128
SCALER = 256.0
NB = 512            # query-block (free dim of score tiles)
NT = N // NB        # 8 query blocks
MT = N // 128       # 32 key tiles of 128
P = 128


def _build():
    import concourse.bass as bass
    import concourse.tile as tile
    from concourse import bacc, mybir
    from concourse.masks import make_identity
    from contextlib import ExitStack

    f32 = mybir.dt.float32
    f16 = mybir.dt.float16
    bf16 = mybir.dt.bfloat16
    AF = mybir.ActivationFunctionType

    nc = bacc.Bacc(
        "TRN2", target_bir_lowering=False, debug=False, num_devices=B
    )

    xT_ext = nc.declare_dram_parameter("xT", [L, N], f32, isOutput=False)
    wq_ext = nc.declare_dram_parameter("Wq", [L, H], f32, isOutput=False)
    wk_ext = nc.declare_dram_parameter("Wk", [L, H], f32, isOutput=False)
    vdT_ext = nc.declare_dram_parameter("VdT", [H, L], f32, isOutput=False)
    vu_ext = nc.declare_dram_parameter("Vu", [H, L], f32, isOutput=False)
    out_ext = nc.declare_dram_parameter("out", [N, L], f32, isOutput=True)

    with tile.TileContext(nc) as tc, ExitStack() as ctx:
        persist = ctx.enter_context(tc.tile_pool(name="persist", bufs=1))

        # constants
        ones_bf = persist.tile([P, 1], bf16)
        nc.gpsimd.memset(ones_bf[:], 1.0)
        ident_f32 = persist.tile([P, P], f32)
        make_identity(nc, ident_f32[:])
        ident_bf = persist.tile([P, P], bf16)
        nc.vector.tensor_copy(ident_bf[:], ident_f32[:])
        one11_f32 = persist.tile([1, 1], f32)
        nc.gpsimd.memset(one11_f32[:], 1.0)

        # persistent fp16/bf16 operands
        qw16 = persist.tile([P, 2 * H], f16)    # Q   [l_chunk][l_in, h]
        kw16 = persist.tile([P, 2 * H], f16)
        vdT16 = persist.tile([P, L], f16)       # V_down.T  [h, l']
        vu16 = persist.tile([P, L], f16)        # V_up      [h, l]
        V16 = persist.tile([P, 2 * L], f16)     # V_down@V_up  [l'_chunk][l'_in, l]
        xt16 = persist.tile([P, 2 * N], f16)    # xT        [l_chunk][l_in, n]
        qT16 = persist.tile([P, N], f16)        # q.T       [h, n]
        kT16 = persist.tile([P, N], f16)        # k.T       [h, m]
        v_sb = persist.tile([P, MT * L], bf16)  # v         [m_tile][m_in, l]

        # ---------------- phase A: loads + casts ----------------
        # Exact-size tags with bufs == allocation count: no slot reuse, so
        # every HWDGE DMA carries at most one semaphore wait (HW limit).
        with tc.tile_pool(name="stage", bufs=8) as stage:
            for c in range(2):
                st = stage.tile([P, H], f32, tag="stw")
                nc.sync.dma_start(st[:], wq_ext[c * P:(c + 1) * P, :])
                nc.vector.tensor_copy(qw16[:, c * H:(c + 1) * H], st[:])
                st = stage.tile([P, H], f32, tag="stw")
                nc.sync.dma_start(st[:], wk_ext[c * P:(c + 1) * P, :])
                nc.vector.tensor_copy(kw16[:, c * H:(c + 1) * H], st[:])
            st = stage.tile([P, L], f32, tag="stv")
            nc.sync.dma_start(st[:], vdT_ext[:, :])
            nc.vector.tensor_copy(vdT16[:], st[:])
            st = stage.tile([P, L], f32, tag="stv")
            nc.sync.dma_start(st[:], vu_ext[:, :])
            nc.vector.tensor_copy(vu16[:], st[:])

            for c in range(2):
                for s in range(4):
                    st = stage.tile([P, 1024], f32, tag="stx")
                    nc.sync.dma_start(
                        st[:], xT_ext[c * P:(c + 1) * P, s * 1024:(s + 1) * 1024]
                    )
                    nc.vector.tensor_copy(
                        xt16[:, c * N + s * 1024: c * N + (s + 1) * 1024], st[:]
                    )

            # ---------------- phase B: V, qT, kT, v ----------------
            with tc.tile_pool(name="pp", bufs=3, space="PSUM") as pp:
                # V = V_down @ V_up : lhsT=V_down.T chunk [h, l'], rhs=V_up [h, l]
                for c in range(2):
                    ps = pp.tile([P, L], f32, tag="pp")
                    nc.tensor.matmul(
                        ps[:], vdT16[:, c * P:(c + 1) * P], vu16[:],
                        start=True, stop=True,
                    )
                    nc.vector.tensor_copy(V16[:, c * L:(c + 1) * L], ps[:])

                # qT[h, n] / kT[h, m]
                for w16, dst in ((qw16, qT16), (kw16, kT16)):
                    for f in range(N // NB):
                        ps = pp.tile([P, NB], f32, tag="pp")
                        for c in range(2):
                            nc.tensor.matmul(
                                ps[:],
                                w16[:, c * H:(c + 1) * H],
                                xt16[:, c * N + f * NB: c * N + (f + 1) * NB],
                                start=(c == 0), stop=(c == 1),
                            )
                        nc.vector.tensor_copy(dst[:, f * NB:(f + 1) * NB], ps[:])

                # v[m, l] : lhsT = xT chunk [l', m_tile], rhs = V chunk [l', l]
                for mt in range(MT):
                    ps = pp.tile([P, L], f32, tag="pp")
                    for c in range(2):
                        nc.tensor.matmul(
                            ps[:],
                            xt16[:, c * N + mt * P: c * N + (mt + 1) * P],
                            V16[:, c * L:(c + 1) * L],
                            start=(c == 0), stop=(c == 1),
                        )
                    nc.vector.tensor_copy(v_sb[:, mt * L:(mt + 1) * L], ps[:])

        # ---------------- phase C: attention main loop ----------------
        with (
            tc.tile_pool(name="est", bufs=2 * MT) as est_pool,
            tc.tile_pool(name="sb_small", bufs=16) as sb_small,
            tc.tile_pool(name="outfin", bufs=NT * (NB // P)) as outfin_pool,
            tc.tile_pool(name="stp", bufs=3, space="PSUM") as stp,
            tc.tile_pool(name="otp", bufs=2, space="PSUM") as otp,
            tc.tile_pool(name="rsp", bufs=1, space="PSUM") as rsp,
            tc.tile_pool(name="tpp", bufs=2, space="PSUM") as tpp,
        ):
            est = {}      # (k, mt) -> bf16 [128, NB] exp score tiles
            otiles = {}   # k -> [lt] psum numerator tiles
            inv_sb = {}   # (k, s) -> [128, 1] f32 inverse rowsums

            for k in range(NT + 1):
                # --- QK + exp for block k
                if k < NT:
                    for mt in range(MT):
                        ps = stp.tile([P, NB], f32, tag="stp")
                        nc.tensor.matmul(
                            ps[:],
                            kT16[:, mt * P:(mt + 1) * P],
                            qT16[:, k * NB:(k + 1) * NB],
                            start=True, stop=True,
                        )
                        e = est_pool.tile([P, NB], bf16, tag="est")
                        est[(k, mt)] = e
                        nc.scalar.activation(
                            e[:], ps[:], AF.Exp, scale=1.0 / SCALER
                        )

                # --- PV for block k-1 (software-pipelined)
                if k >= 1:
                    ots = []
                    for lt in range(2):
                        ot = otp.tile([P, NB], f32, tag="otp")
                        ots.append(ot)
                        for mt in range(MT):
                            nc.tensor.matmul(
                                ot[:],
                                v_sb[:, mt * L + lt * P: mt * L + (lt + 1) * P],
                                est[(k - 1, mt)][:],
                                start=(mt == 0), stop=(mt == MT - 1),
                            )
                    otiles[k - 1] = ots

                # --- rowsums for block k (ones-vector matmul over partitions)
                if k < NT:
                    rs = rsp.tile([1, NB], f32, tag="rsp")
                    for mt in range(MT):
                        nc.tensor.matmul(
                            rs[:], ones_bf[:], est[(k, mt)][:],
                            start=(mt == 0), stop=(mt == MT - 1),
                        )
                    rsinv = sb_small.tile([1, NB], f32, tag="rsinv")
                    nc.vector.reciprocal(rsinv[:], rs[:])
                    for s in range(NB // P):
                        tp = tpp.tile([P, 1], f32, tag="tp")
                        nc.tensor.transpose(
                            tp[:], rsinv[:, s * P:(s + 1) * P], one11_f32[:]
                        )
                        iv = sb_small.tile([P, 1], f32, tag="inv")
                        inv_sb[(k, s)] = iv
                        nc.vector.tensor_copy(iv[:], tp[:])

                # --- drain block k-1: transpose numerator, scale, store
                if k >= 1:
                    osb = []
                    for lt in range(2):
                        ob = sb_small.tile([P, NB], bf16, tag="outT")
                        nc.vector.tensor_copy(ob[:], otiles[k - 1][lt][:])
                        osb.append(ob)
                    for s in range(NB // P):
                        fin = outfin_pool.tile([P, L], f32, tag="fin")
                        for lt in range(2):
                            tp2 = tpp.tile([P, P], bf16, tag="tp")
                            nc.tensor.transpose(
                                tp2[:], osb[lt][:, s * P:(s + 1) * P], ident_bf[:]
                            )
                            nc.scalar.activation(
                                fin[:, lt * P:(lt + 1) * P], tp2[:],
                                AF.Copy, scale=inv_sb[(k - 1, s)][:],
                            )
                        r0 = (k - 1) * NB + s * P
                        nc.gpsimd.dma_start(out_ext[r0:r0 + P, :], fin[:])

    if not nc.is_finalized():
        nc.finalize()
    return nc


_GRAPH_CACHE = {}


def _get_graph():
    if "nc" not in _GRAPH_CACHE:
        _GRAPH_CACHE["nc"] = _build()
    return _GRAPH_CACHE["nc"]


def run(inputs: dict, trace: bool = False):
    """Run the SPMD kernel on 8 cores. Returns (output, BassKernelResults)."""
    from concourse.bass_utils import run_bass_kernel_spmd

    x = np.asarray(inputs["x"], dtype=np.float32)
    Q = np.asarray(inputs["Q"], dtype=np.float32)[0]
    K = np.asarray(inputs["K"], dtype=np.float32)[0]
    Vd = np.asarray(inputs["V_down"], dtype=np.float32)[0]
    Vu = np.asarray(inputs["V_up"], dtype=np.float32)[0]

    wq = np.ascontiguousarray(Q)
    wk = np.ascontiguousarray(K)
    vdT = np.ascontiguousarray(Vd.T)
    vu = np.ascontiguousarray(Vu)

    in_maps = []
    for b in range(B):
        in_maps.append({
            "xT": np.ascontiguousarray(x[b].T),
            "Wq": wq,
            "Wk": wk,
            "VdT": vdT,
            "Vu": vu,
        })

    nc = _get_graph()
    res = run_bass_kernel_spmd(nc, in_maps, core_ids=list(range(B)), trace=trace)
    out = np.stack([np.asarray(res.results[i]["out"]) for i in range(B)])
    return out.astype(np.float32), res


def kernel(**inputs) -> np.ndarray:
    out, _ = run(inputs, trace=False)
    return out


# revision 13
# speedup vs baseline: 1.1493x; 1.1493x over previous
"""Trainium2 Bass kernel for nn_Attention_27358941675773.

Reference computation (per batch b):
    q = x @ Q              [N, H]
    k = x @ K              [N, H]
    V = V_down @ V_up      [L, L]
    v = x @ V              [N, L]
    S = q @ k.T / 256      [N, N]
    out = softmax(S) @ v   [N, L]

Sharding: pure data-parallel over batch B=8 across the 8 NeuronCores
(one batch element per core); small params replicated. No collectives.

Per-core kernel strategy (N=4096, L=256, H=128):
  - All projection inputs cast to fp16 on-chip; matmuls run at full PE
    rate (1 cyc/row). qT [H,N] and kT [H,N] are computed directly in
    transposed layout so that scores can be built as S_T[m, n] (keys on
    partitions) without any transposes.
  - exp(S_T/256) is computed on the Scalar engine straight out of PSUM,
    written as bf16 (scores can reach ~±70; exp stays in fp32/bf16
    range, so no max-subtraction pass is needed).
  - softmax denominator rowsum[n] = sum_m exp(S_T[m,n]) via a ones-vector
    matmul accumulated in PSUM (reduction over the partition axis).
  - numerator out^T[l, n] = sum_m v[m, l] * expS_T[m, n] via matmul with
    v kept in its natural [m, l] layout - again no transposes.
  - final: out^T tiles are PE-transposed back to [n, l], scaled by
    1/rowsum (per-partition scalar on the Scalar engine) and DMA'd out.
  - The PV matmul of block k-1 is software-pipelined against the
    QK/exp of block k so the Scalar engine's exp stream stays hidden.
"""

import os
import sys

import numpy as np

for _p in ("/opt/trn_rl_repo",):
    if _p not in sys.path and os.path.isdir(_p):
        sys.path.insert(0, _p)

B, N, L, H = 8, 4096, 256, 128
SCALER = 256.0
NB = 512            # query-block (free dim of score tiles)
NT = N // NB        # 8 query blocks
MT = N // 128       # 32 key tiles of 128
P = 128


def _build():
    import concourse.bass as bass
    import concourse.tile as tile
    from concourse import bacc, mybir
    from concourse.masks import make_identity
    from contextlib import ExitStack

    f32 = mybir.dt.float32
    f16 = mybir.dt.float16
    bf16 = mybir.dt.bfloat16
    AF = mybir.ActivationFunctionType

    nc = bacc.Bacc(
        "TRN2", target_bir_lowering=False, debug=False, num_devices=B
    )

    xT_ext = nc.declare_dram_parameter("xT", [L, N], f32, isOutput=False)
    wq_ext = nc.declare_dram_parameter("Wq", [L, H], f32, isOutput=False)
    wk_ext = nc.declare_dram_parameter("Wk", [L, H], f32, isOutput=False)
    vdT_ext = nc.declare_dram_parameter("VdT", [H, L], f32, isOutput=False)
    vu_ext = nc.declare_dram_parameter("Vu", [H, L], f32, isOutput=False)
    out_ext = nc.declare_dram_parameter("out", [N, L], f32, isOutput=True)

    with tile.TileContext(nc) as tc, ExitStack() as ctx:
        persist = ctx.enter_context(tc.tile_pool(name="persist", bufs=1))

        # constants
        ones_bf = persist.tile([P, 1], bf16)
        nc.gpsimd.memset(ones_bf[:], 1.0)
        ident_f32 = persist.tile([P, P], f32)
        make_identity(nc, ident_f32[:])
        ident_bf = persist.tile([P, P], bf16)
        nc.vector.tensor_copy(ident_bf[:], ident_f32[:])
        one11_f32 = persist.tile([1, 1], f32)
        nc.gpsimd.memset(one11_f32[:], 1.0)

        # persistent fp16/bf16 operands
        qw16 = persist.tile([P, 2 * H], f16)    # Q   [l_chunk][l_in, h]
        kw16 = persist.tile([P, 2 * H], f16)
        vdT16 = persist.tile([P, L], f16)       # V_down.T  [h, l']
        vu16 = persist.tile([P, L], f16)        # V_up      [h, l]
        V16 = persist.tile([P, 2 * L], f16)     # V_down@V_up  [l'_chunk][l'_in, l]
        # xT fp16, split into 8 chunk tiles [l_chunk][n_1024_chunk] so the
        # projection matmuls can start as soon as their chunks have landed.
        xt16 = [
            [
                persist.tile(
                    [P, 1024], f16, tag=f"xt{c}_{s}", name=f"xt16_{c}_{s}"
                )
                for s in range(4)
            ]
            for c in range(2)
        ]
        qT16 = persist.tile([P, N], f16)        # q.T       [h, n]
        kT16 = persist.tile([P, N], f16)        # k.T       [h, m]
        v_sb = persist.tile([P, MT * L], bf16)  # v         [m_tile][m_in, l]

        # ---------------- phase A: loads + casts ----------------
        # Exact-size tags with bufs == allocation count: no slot reuse, so
        # every HWDGE DMA carries at most one semaphore wait (HW limit).
        with tc.tile_pool(name="stage", bufs=8) as stage:
            for c in range(2):
                st = stage.tile([P, H], f32, tag="stw")
                nc.sync.dma_start(st[:], wq_ext[c * P:(c + 1) * P, :])
                nc.vector.tensor_copy(qw16[:, c * H:(c + 1) * H], st[:])
                st = stage.tile([P, H], f32, tag="stw")
                nc.sync.dma_start(st[:], wk_ext[c * P:(c + 1) * P, :])
                nc.vector.tensor_copy(kw16[:, c * H:(c + 1) * H], st[:])
            st = stage.tile([P, L], f32, tag="stv")
            nc.sync.dma_start(st[:], vdT_ext[:, :])
            nc.vector.tensor_copy(vdT16[:], st[:])
            st = stage.tile([P, L], f32, tag="stv")
            nc.sync.dma_start(st[:], vu_ext[:, :])
            nc.vector.tensor_copy(vu16[:], st[:])

            # interleave the two l_chunks so both chunks of an n-range land
            # early and the dependent projection matmuls can start
            for s in range(4):
                for c in range(2):
                    st = stage.tile([P, 1024], f32, tag="stx")
                    nc.sync.dma_start(
                        st[:], xT_ext[c * P:(c + 1) * P, s * 1024:(s + 1) * 1024]
                    )
                    nc.vector.tensor_copy(xt16[c][s][:], st[:])

            # ---------------- phase B: V, qT, kT, v ----------------
            with tc.tile_pool(name="pp", bufs=3, space="PSUM") as pp:
                # V = V_down @ V_up : lhsT=V_down.T chunk [h, l'], rhs=V_up [h, l]
                for c in range(2):
                    ps = pp.tile([P, L], f32, tag="pp")
                    nc.tensor.matmul(
                        ps[:], vdT16[:, c * P:(c + 1) * P], vu16[:],
                        start=True, stop=True,
                    )
                    nc.vector.tensor_copy(V16[:, c * L:(c + 1) * L], ps[:])

                # qT[h, n] / kT[h, m]
                for w16, dst in ((qw16, qT16), (kw16, kT16)):
                    for f in range(N // NB):
                        ps = pp.tile([P, NB], f32, tag="pp")
                        for c in range(2):
                            nc.tensor.matmul(
                                ps[:],
                                w16[:, c * H:(c + 1) * H],
                                xt16[c][f // 2][:, (f % 2) * NB:(f % 2 + 1) * NB],
                                start=(c == 0), stop=(c == 1),
                            )
                        nc.vector.tensor_copy(dst[:, f * NB:(f + 1) * NB], ps[:])

                # v[m, l] : lhsT = xT chunk [l', m_tile], rhs = V chunk [l', l]
                for mt in range(MT):
                    ps = pp.tile([P, L], f32, tag="pp")
                    for c in range(2):
                        nc.tensor.matmul(
                            ps[:],
                            xt16[c][mt // 8][:, (mt % 8) * P:(mt % 8 + 1) * P],
                            V16[:, c * L:(c + 1) * L],
                            start=(c == 0), stop=(c == 1),
                        )
                    nc.vector.tensor_copy(v_sb[:, mt * L:(mt + 1) * L], ps[:])

        # ---------------- phase C: attention main loop ----------------
        with (
            tc.tile_pool(name="est", bufs=2 * MT) as est_pool,
            tc.tile_pool(name="sb_small", bufs=4) as sb_small,
            tc.tile_pool(name="outfin", bufs=6) as outfin_pool,
            tc.tile_pool(name="stp", bufs=3, space="PSUM") as stp,
            tc.tile_pool(name="otp", bufs=2, space="PSUM") as otp,
            tc.tile_pool(name="rsp", bufs=1, space="PSUM") as rsp,
            tc.tile_pool(name="tpp", bufs=2, space="PSUM") as tpp,
        ):
            est = {}      # (k, mt) -> bf16 [128, NB] exp score tiles
            otiles = {}   # k -> [lt] psum numerator tiles
            inv_sb = {}   # (k, s) -> [128, 1] f32 inverse rowsums

            for k in range(NT + 1):
                # --- QK + exp for block k
                if k < NT:
                    for mt in range(MT):
                        ps = stp.tile([P, NB], f32, tag="stp")
                        nc.tensor.matmul(
                            ps[:],
                            kT16[:, mt * P:(mt + 1) * P],
                            qT16[:, k * NB:(k + 1) * NB],
                            start=True, stop=True,
                        )
                        e = est_pool.tile([P, NB], bf16, tag="est")
                        est[(k, mt)] = e
                        nc.scalar.activation(
                            e[:], ps[:], AF.Exp, scale=1.0 / SCALER
                        )

                # --- PV for block k-1 (software-pipelined)
                if k >= 1:
                    ots = []
                    for lt in range(2):
                        ot = otp.tile([P, NB], f32, tag="otp")
                        ots.append(ot)
                        for mt in range(MT):
                            nc.tensor.matmul(
                                ot[:],
                                v_sb[:, mt * L + lt * P: mt * L + (lt + 1) * P],
                                est[(k - 1, mt)][:],
                                start=(mt == 0), stop=(mt == MT - 1),
                            )
                    otiles[k - 1] = ots

                # --- rowsums for block k: two levels of pairwise adds on the
                # (otherwise idle) Vector engine, then an 8-chunk ones-vector
                # matmul over the partition axis on PE.
                if k < NT:
                    t2 = []
                    for j in range(MT // 4):
                        t1a = sb_small.tile([P, NB], bf16, tag="t1", bufs=4)
                        nc.vector.tensor_add(
                            t1a[:], est[(k, 4 * j)][:], est[(k, 4 * j + 1)][:]
                        )
                        t1b = sb_small.tile([P, NB], bf16, tag="t1", bufs=4)
                        nc.vector.tensor_add(
                            t1b[:], est[(k, 4 * j + 2)][:], est[(k, 4 * j + 3)][:]
                        )
                        t2j = sb_small.tile([P, NB], bf16, tag="t2", bufs=10)
                        nc.vector.tensor_add(t2j[:], t1a[:], t1b[:])
                        t2.append(t2j)
                    rs = rsp.tile([1, NB], f32, tag="rsp")
                    for j in range(MT // 4):
                        nc.tensor.matmul(
                            rs[:], ones_bf[:], t2[j][:],
                            start=(j == 0), stop=(j == MT // 4 - 1),
                        )
                    rsinv = sb_small.tile([1, NB], f32, tag="rsinv", bufs=2)
                    nc.vector.reciprocal(rsinv[:], rs[:])
                    for s in range(NB // P):
                        tp = tpp.tile([P, 1], f32, tag="tp")
                        nc.tensor.transpose(
                            tp[:], rsinv[:, s * P:(s + 1) * P], one11_f32[:]
                        )
                        iv = sb_small.tile([P, 1], f32, tag="inv", bufs=8)
                        inv_sb[(k, s)] = iv
                        nc.vector.tensor_copy(iv[:], tp[:])

                # --- drain block k-1: transpose numerator, scale, store
                if k >= 1:
                    osb = []
                    for lt in range(2):
                        ob = sb_small.tile([P, NB], bf16, tag="outT", bufs=4)
                        nc.vector.tensor_copy(ob[:], otiles[k - 1][lt][:])
                        osb.append(ob)
                    for s in range(NB // P):
                        fin = outfin_pool.tile([P, L], f32, tag="fin")
                        for lt in range(2):
                            tp2 = tpp.tile([P, P], bf16, tag="tp")
                            nc.tensor.transpose(
                                tp2[:], osb[lt][:, s * P:(s + 1) * P], ident_bf[:]
                            )
                            nc.vector.tensor_scalar_mul(
                                fin[:, lt * P:(lt + 1) * P], tp2[:],
                                inv_sb[(k - 1, s)][:],
                            )
                        r0 = (k - 1) * NB + s * P
                        nc.gpsimd.dma_start(out_ext[r0:r0 + P, :], fin[:])

    if not nc.is_finalized():
        nc.finalize()
    return nc


_GRAPH_CACHE = {}


def _get_graph():
    if "nc" not in _GRAPH_CACHE:
        _GRAPH_CACHE["nc"] = _build()
    return _GRAPH_CACHE["nc"]


def run(inputs: dict, trace: bool = False):
    """Run the SPMD kernel on 8 cores. Returns (output, BassKernelResults)."""
    from concourse.bass_utils import run_bass_kernel_spmd

    x = np.asarray(inputs["x"], dtype=np.float32)
    Q = np.asarray(inputs["Q"], dtype=np.float32)[0]
    K = np.asarray(inputs["K"], dtype=np.float32)[0]
    Vd = np.asarray(inputs["V_down"], dtype=np.float32)[0]
    Vu = np.asarray(inputs["V_up"], dtype=np.float32)[0]

    wq = np.ascontiguousarray(Q)
    wk = np.ascontiguousarray(K)
    vdT = np.ascontiguousarray(Vd.T)
    vu = np.ascontiguousarray(Vu)

    in_maps = []
    for b in range(B):
        in_maps.append({
            "xT": np.ascontiguousarray(x[b].T),
            "Wq": wq,
            "Wk": wk,
            "VdT": vdT,
            "Vu": vu,
        })

    nc = _get_graph()
    res = run_bass_kernel_spmd(nc, in_maps, core_ids=list(range(B)), trace=trace)
    out = np.stack([np.asarray(res.results[i]["out"]) for i in range(B)])
    return out.astype(np.float32), res


def kernel(**inputs) -> np.ndarray:
    out, _ = run(inputs, trace=False)
    return out


# revision 14
# speedup vs baseline: 1.3706x; 1.1925x over previous
"""Trainium2 Bass kernel for nn_Attention_27358941675773.

Reference computation (per batch b):
    q = x @ Q              [N, H]
    k = x @ K              [N, H]
    V = V_down @ V_up      [L, L]
    v = x @ V              [N, L]
    S = q @ k.T / 256      [N, N]
    out = softmax(S) @ v   [N, L]

Sharding: pure data-parallel over batch B=8 across the 8 NeuronCores
(one batch element per core); small params replicated. No collectives.

Per-core kernel strategy (N=4096, L=256, H=128):
  - Inputs are shipped as fp16 (x transposed to [L, N]); all matmuls run
    at full PE rate (1 cyc/row). qT [H,N] and kT [H,N] are computed
    directly in transposed layout so scores can be built as S_T[m, n]
    (keys on partitions) without any transposes.
  - exp(S_T/256) runs on the Scalar engine straight out of PSUM, written
    as bf16 (scores reach ~±70; exp stays in fp32/bf16 range, so no
    max-subtraction pass is needed).
  - softmax denominator rowsum[n] = sum_m exp(S_T[m,n]): two levels of
    pairwise adds on the Vector engine, then an 8-chunk ones-vector
    matmul accumulated in PSUM (partition-axis reduction).
  - numerator out^T[l, n] = sum_m v[m, l] * expS_T[m, n] via matmul with
    v kept in its natural [m, l] layout - no transposes.
  - normalization: 1/rowsum is partition-broadcast on GpSimd and applied
    with one Vector multiply; the output is stored TRANSPOSED [L, N] in
    DRAM and un-transposed on the host during the gather.
  - The PV matmul of block k-1 is software-pipelined against the QK/exp
    of block k so the Scalar engine's exp stream stays hidden.
"""

import os
import sys

import numpy as np

for _p in ("/opt/trn_rl_repo",):
    if _p not in sys.path and os.path.isdir(_p):
        sys.path.insert(0, _p)

B, N, L, H = 8, 4096, 256, 128
SCALER = 256.0
NB = 512            # query-block (free dim of score tiles)
NT = N // NB        # 8 query blocks
MT = N // 128       # 32 key tiles of 128
P = 128


def _build():
    import concourse.bass as bass
    import concourse.tile as tile
    from concourse import bacc, mybir
    from contextlib import ExitStack

    f32 = mybir.dt.float32
    f16 = mybir.dt.float16
    bf16 = mybir.dt.bfloat16
    AF = mybir.ActivationFunctionType

    nc = bacc.Bacc(
        "TRN2", target_bir_lowering=False, debug=False, num_devices=B
    )

    xT_ext = nc.declare_dram_parameter("xT", [L, N], f16, isOutput=False)
    wq_ext = nc.declare_dram_parameter("Wq", [L, H], f16, isOutput=False)
    wk_ext = nc.declare_dram_parameter("Wk", [L, H], f16, isOutput=False)
    vdT_ext = nc.declare_dram_parameter("VdT", [H, L], f16, isOutput=False)
    vu_ext = nc.declare_dram_parameter("Vu", [H, L], f16, isOutput=False)
    # output stored transposed [L, N]; host un-transposes at gather
    out_ext = nc.declare_dram_parameter("out", [L, N], f32, isOutput=True)

    with tile.TileContext(nc) as tc, ExitStack() as ctx:
        persist = ctx.enter_context(tc.tile_pool(name="persist", bufs=1))

        ones_bf = persist.tile([P, 1], bf16)
        nc.gpsimd.memset(ones_bf[:], 1.0)

        qw16 = persist.tile([P, 2 * H], f16)    # Q   [l_chunk][l_in, h]
        kw16 = persist.tile([P, 2 * H], f16)
        vdT16 = persist.tile([P, L], f16)       # V_down.T  [h, l']
        vu16 = persist.tile([P, L], f16)        # V_up      [h, l]
        V16 = persist.tile([P, 2 * L], f16)     # V_down@V_up  [l'_chunk][l'_in, l]
        xt16 = [
            [
                persist.tile(
                    [P, 1024], f16, tag=f"xt{c}_{s}", name=f"xt16_{c}_{s}"
                )
                for s in range(4)
            ]
            for c in range(2)
        ]
        qT16 = persist.tile([P, N], f16)        # q.T       [h, n]
        kT16 = persist.tile([P, N], f16)        # k.T       [h, m]
        v_sb = persist.tile([P, MT * L], bf16)  # v         [m_tile][m_in, l]

        # ---------------- phase A: direct fp16 loads ----------------
        for c in range(2):
            nc.sync.dma_start(qw16[:, c * H:(c + 1) * H], wq_ext[c * P:(c + 1) * P, :])
            nc.sync.dma_start(kw16[:, c * H:(c + 1) * H], wk_ext[c * P:(c + 1) * P, :])
        nc.sync.dma_start(vdT16[:], vdT_ext[:, :])
        nc.sync.dma_start(vu16[:], vu_ext[:, :])
        # interleave the two l_chunks so both chunks of an n-range land early
        for s in range(4):
            for c in range(2):
                nc.sync.dma_start(
                    xt16[c][s][:],
                    xT_ext[c * P:(c + 1) * P, s * 1024:(s + 1) * 1024],
                )

        # ---------------- phase B: V, qT, kT, v ----------------
        with tc.tile_pool(name="pp", bufs=3, space="PSUM") as pp:
            # V = V_down @ V_up : lhsT=V_down.T chunk [h, l'], rhs=V_up [h, l]
            for c in range(2):
                ps = pp.tile([P, L], f32, tag="pp")
                nc.tensor.matmul(
                    ps[:], vdT16[:, c * P:(c + 1) * P], vu16[:],
                    start=True, stop=True,
                )
                nc.vector.tensor_copy(V16[:, c * L:(c + 1) * L], ps[:])

            # qT[h, n] / kT[h, m]
            for w16, dst in ((kw16, kT16), (qw16, qT16)):
                for f in range(N // NB):
                    ps = pp.tile([P, NB], f32, tag="pp")
                    for c in range(2):
                        nc.tensor.matmul(
                            ps[:],
                            w16[:, c * H:(c + 1) * H],
                            xt16[c][f // 2][:, (f % 2) * NB:(f % 2 + 1) * NB],
                            start=(c == 0), stop=(c == 1),
                        )
                    nc.vector.tensor_copy(dst[:, f * NB:(f + 1) * NB], ps[:])

            # v[m, l] : lhsT = xT chunk [l', m_tile], rhs = V chunk [l', l]
            for mt in range(MT):
                ps = pp.tile([P, L], f32, tag="pp")
                for c in range(2):
                    nc.tensor.matmul(
                        ps[:],
                        xt16[c][mt // 8][:, (mt % 8) * P:(mt % 8 + 1) * P],
                        V16[:, c * L:(c + 1) * L],
                        start=(c == 0), stop=(c == 1),
                    )
                nc.vector.tensor_copy(v_sb[:, mt * L:(mt + 1) * L], ps[:])

        # ---------------- phase C: attention main loop ----------------
        with (
            tc.tile_pool(name="est", bufs=2 * MT) as est_pool,
            tc.tile_pool(name="sb_small", bufs=4) as sb_small,
            tc.tile_pool(name="outfin", bufs=4) as outfin_pool,
            tc.tile_pool(name="stp", bufs=4, space="PSUM") as stp,
            tc.tile_pool(name="otp", bufs=3, space="PSUM") as otp,
            tc.tile_pool(name="rsp", bufs=1, space="PSUM") as rsp,
        ):
            est = {}      # (k, mt) -> bf16 [128, NB] exp score tiles
            otiles = {}   # k -> [lt] psum numerator tiles
            bc = {}       # k -> [128, NB] f32 broadcast 1/rowsum

            for k in range(NT + 1):
                # --- QK + exp for block k
                if k < NT:
                    for mt in range(MT):
                        ps = stp.tile([P, NB], f32, tag="stp")
                        nc.tensor.matmul(
                            ps[:],
                            kT16[:, mt * P:(mt + 1) * P],
                            qT16[:, k * NB:(k + 1) * NB],
                            start=True, stop=True,
                        )
                        e = est_pool.tile([P, NB], bf16, tag="est")
                        est[(k, mt)] = e
                        nc.scalar.activation(
                            e[:], ps[:], AF.Exp, scale=1.0 / SCALER
                        )

                # --- PV for block k-1 (software-pipelined)
                if k >= 1:
                    ots = []
                    for lt in range(2):
                        ot = otp.tile([P, NB], f32, tag="otp")
                        ots.append(ot)
                        for mt in range(MT):
                            nc.tensor.matmul(
                                ot[:],
                                v_sb[:, mt * L + lt * P: mt * L + (lt + 1) * P],
                                est[(k - 1, mt)][:],
                                start=(mt == 0), stop=(mt == MT - 1),
                            )
                    otiles[k - 1] = ots

                # --- rowsums for block k: two levels of pairwise adds on the
                # Vector engine, then an 8-chunk ones matmul over partitions.
                if k < NT:
                    t2 = []
                    for j in range(MT // 4):
                        t1a = sb_small.tile([P, NB], bf16, tag="t1", bufs=4)
                        nc.vector.tensor_add(
                            t1a[:], est[(k, 4 * j)][:], est[(k, 4 * j + 1)][:]
                        )
                        t1b = sb_small.tile([P, NB], bf16, tag="t1", bufs=4)
                        nc.vector.tensor_add(
                            t1b[:], est[(k, 4 * j + 2)][:], est[(k, 4 * j + 3)][:]
                        )
                        t2j = sb_small.tile([P, NB], bf16, tag="t2", bufs=10)
                        nc.vector.tensor_add(t2j[:], t1a[:], t1b[:])
                        t2.append(t2j)
                    rs = rsp.tile([1, NB], f32, tag="rsp")
                    for j in range(MT // 4):
                        nc.tensor.matmul(
                            rs[:], ones_bf[:], t2[j][:],
                            start=(j == 0), stop=(j == MT // 4 - 1),
                        )
                    rsinv = sb_small.tile([1, NB], f32, tag="rsinv", bufs=2)
                    nc.vector.reciprocal(rsinv[:], rs[:])
                    bck = sb_small.tile([P, NB], f32, tag="bc", bufs=2)
                    nc.gpsimd.partition_broadcast(bck[:], rsinv[:])
                    bc[k] = bck

                # --- drain block k-1: scale by 1/rowsum, store transposed
                if k >= 1:
                    for lt in range(2):
                        fin = outfin_pool.tile([P, NB], f32, tag="fin")
                        nc.vector.tensor_mul(
                            fin[:], otiles[k - 1][lt][:], bc[k - 1][:]
                        )
                        nc.gpsimd.dma_start(
                            out_ext[lt * P:(lt + 1) * P, (k - 1) * NB:k * NB],
                            fin[:],
                        )

    if not nc.is_finalized():
        nc.finalize()
    return nc


_GRAPH_CACHE = {}


def _get_graph():
    if "nc" not in _GRAPH_CACHE:
        _GRAPH_CACHE["nc"] = _build()
    return _GRAPH_CACHE["nc"]


def run(inputs: dict, trace: bool = False):
    """Run the SPMD kernel on 8 cores. Returns (output, BassKernelResults)."""
    from concourse.bass_utils import run_bass_kernel_spmd

    x = np.asarray(inputs["x"], dtype=np.float32)
    Q = np.asarray(inputs["Q"], dtype=np.float32)[0]
    K = np.asarray(inputs["K"], dtype=np.float32)[0]
    Vd = np.asarray(inputs["V_down"], dtype=np.float32)[0]
    Vu = np.asarray(inputs["V_up"], dtype=np.float32)[0]

    wq = np.ascontiguousarray(Q).astype(np.float16)
    wk = np.ascontiguousarray(K).astype(np.float16)
    vdT = np.ascontiguousarray(Vd.T).astype(np.float16)
    vu = np.ascontiguousarray(Vu).astype(np.float16)

    in_maps = []
    for b in range(B):
        in_maps.append({
            "xT": np.ascontiguousarray(x[b].T).astype(np.float16),
            "Wq": wq,
            "Wk": wk,
            "VdT": vdT,
            "Vu": vu,
        })

    nc = _get_graph()
    res = run_bass_kernel_spmd(nc, in_maps, core_ids=list(range(B)), trace=trace)
    # device output is [L, N] per core; un-transpose during the gather
    out = np.stack([np.asarray(res.results[i]["out"]).T for i in range(B)])
    return np.ascontiguousarray(out, dtype=np.float32), res


def kernel(**inputs) -> np.ndarray:
    out, _ = run(inputs, trace=False)
    return out


# revision 15
# speedup vs baseline: 1.3901x; 1.0142x over previous
"""Trainium2 Bass kernel for nn_Attention_27358941675773.

Reference computation (per batch b):
    q = x @ Q              [N, H]
    k = x @ K              [N, H]
    V = V_down @ V_up      [L, L]
    v = x @ V              [N, L]
    S = q @ k.T / 256      [N, N]
    out = softmax(S) @ v   [N, L]

Sharding: pure data-parallel over batch B=8 across the 8 NeuronCores
(one batch element per core); small params replicated. No collectives.

Per-core kernel strategy (N=4096, L=256, H=128):
  - Inputs are shipped as fp16 (x transposed to [L, N]); all matmuls run
    at full PE rate (1 cyc/row). qT [H,N] and kT [H,N] are computed
    directly in transposed layout so scores can be built as S_T[m, n]
    (keys on partitions) without any transposes.
  - The value path is factored through the rank-H bottleneck:
        out = softmax(S) @ x @ V_down @ V_up
    so the O(N^2) matmul contracts into H=128 columns (w = x @ V_down),
    and V_up is applied after the softmax-normalization - halving the
    PE work of the attention*value product.
  - exp(S_T/256) runs on the Scalar engine straight out of PSUM, written
    as bf16 (scores reach ~±70; exp stays in fp32/bf16 range, so no
    max-subtraction pass is needed).
  - softmax denominator rowsum[n] = sum_m exp(S_T[m,n]): two levels of
    pairwise adds on the Vector engine, then an 8-chunk ones-vector
    matmul accumulated in PSUM (partition-axis reduction).
  - normalization: 1/rowsum is partition-broadcast on GpSimd and applied
    to mid^T = w^T-weighted numerator with one Vector multiply; the
    output is stored TRANSPOSED [L, N] in DRAM and un-transposed on the
    host during the gather.
  - The attention*w matmul of block k-1 is software-pipelined against
    the QK/exp of block k so the Scalar engine's exp stream stays hidden.
"""

import os
import sys

import numpy as np

for _p in ("/opt/trn_rl_repo",):
    if _p not in sys.path and os.path.isdir(_p):
        sys.path.insert(0, _p)

B, N, L, H = 8, 4096, 256, 128
SCALER = 256.0
NB = 512            # query-block (free dim of score tiles)
NT = N // NB        # 8 query blocks
MT = N // 128       # 32 key tiles of 128
P = 128


def _build():
    import concourse.bass as bass
    import concourse.tile as tile
    from concourse import bacc, mybir
    from contextlib import ExitStack

    f32 = mybir.dt.float32
    f16 = mybir.dt.float16
    bf16 = mybir.dt.bfloat16
    AF = mybir.ActivationFunctionType

    nc = bacc.Bacc(
        "TRN2", target_bir_lowering=False, debug=False, num_devices=B
    )

    xT_ext = nc.declare_dram_parameter("xT", [L, N], f16, isOutput=False)
    wq_ext = nc.declare_dram_parameter("Wq", [L, H], f16, isOutput=False)
    wk_ext = nc.declare_dram_parameter("Wk", [L, H], f16, isOutput=False)
    vd_ext = nc.declare_dram_parameter("Vd", [L, H], f16, isOutput=False)
    vu_ext = nc.declare_dram_parameter("Vu", [H, L], f16, isOutput=False)
    # output stored transposed [L, N]; host un-transposes at gather
    out_ext = nc.declare_dram_parameter("out", [L, N], f32, isOutput=True)

    with tile.TileContext(nc) as tc, ExitStack() as ctx:
        persist = ctx.enter_context(tc.tile_pool(name="persist", bufs=1))

        ones_bf = persist.tile([P, 1], bf16)
        nc.gpsimd.memset(ones_bf[:], 1.0)

        qw16 = persist.tile([P, 2 * H], f16)    # Q   [l_chunk][l_in, h]
        kw16 = persist.tile([P, 2 * H], f16)
        vd16 = persist.tile([P, 2 * H], f16)    # V_down [l_chunk][l_in, h]
        vu16 = persist.tile([P, L], f16)        # V_up   [h, l]
        xt16 = [
            [
                persist.tile(
                    [P, 1024], f16, tag=f"xt{c}_{s}", name=f"xt16_{c}_{s}"
                )
                for s in range(4)
            ]
            for c in range(2)
        ]
        qT16 = persist.tile([P, N], f16)        # q.T       [h, n]
        kT16 = persist.tile([P, N], f16)        # k.T       [h, m]
        w_sb = persist.tile([P, MT * H], bf16)  # x@V_down  [m_tile][m_in, h]

        # ---------------- phase A: direct fp16 loads ----------------
        for c in range(2):
            nc.sync.dma_start(qw16[:, c * H:(c + 1) * H], wq_ext[c * P:(c + 1) * P, :])
            nc.sync.dma_start(kw16[:, c * H:(c + 1) * H], wk_ext[c * P:(c + 1) * P, :])
            nc.sync.dma_start(vd16[:, c * H:(c + 1) * H], vd_ext[c * P:(c + 1) * P, :])
        nc.sync.dma_start(vu16[:], vu_ext[:, :])
        # interleave the two l_chunks so both chunks of an n-range land early
        for s in range(4):
            for c in range(2):
                nc.sync.dma_start(
                    xt16[c][s][:],
                    xT_ext[c * P:(c + 1) * P, s * 1024:(s + 1) * 1024],
                )

        # ---------------- phase B: qT, kT, w ----------------
        with tc.tile_pool(name="pp", bufs=3, space="PSUM") as pp:
            # qT[h, n] / kT[h, m]
            for w16, dst in ((kw16, kT16), (qw16, qT16)):
                for f in range(N // NB):
                    ps = pp.tile([P, NB], f32, tag="pp")
                    for c in range(2):
                        nc.tensor.matmul(
                            ps[:],
                            w16[:, c * H:(c + 1) * H],
                            xt16[c][f // 2][:, (f % 2) * NB:(f % 2 + 1) * NB],
                            start=(c == 0), stop=(c == 1),
                        )
                    nc.vector.tensor_copy(dst[:, f * NB:(f + 1) * NB], ps[:])

            # w[m, h] = x @ V_down : lhsT = xT chunk [l', m_tile]
            for mt in range(MT):
                ps = pp.tile([P, H], f32, tag="ppw")
                for c in range(2):
                    nc.tensor.matmul(
                        ps[:],
                        xt16[c][mt // 8][:, (mt % 8) * P:(mt % 8 + 1) * P],
                        vd16[:, c * H:(c + 1) * H],
                        start=(c == 0), stop=(c == 1),
                    )
                nc.vector.tensor_copy(w_sb[:, mt * H:(mt + 1) * H], ps[:])

        # ---------------- phase C: attention main loop ----------------
        with (
            tc.tile_pool(name="est", bufs=2 * MT) as est_pool,
            tc.tile_pool(name="sb_small", bufs=4) as sb_small,
            tc.tile_pool(name="outfin", bufs=4) as outfin_pool,
            tc.tile_pool(name="stp", bufs=3, space="PSUM") as stp,
            tc.tile_pool(name="mtp", bufs=2, space="PSUM") as mtp,
            tc.tile_pool(name="otp", bufs=2, space="PSUM") as otp,
            tc.tile_pool(name="rsp", bufs=1, space="PSUM") as rsp,
        ):
            est = {}      # (k, mt) -> bf16 [128, NB] exp score tiles
            mtiles = {}   # k -> psum numerator mid^T [h, n] tile
            bc = {}       # k -> [128, NB] f32 broadcast 1/rowsum

            for k in range(NT + 1):
                # --- QK + exp for block k
                if k < NT:
                    for mt in range(MT):
                        ps = stp.tile([P, NB], f32, tag="stp")
                        nc.tensor.matmul(
                            ps[:],
                            kT16[:, mt * P:(mt + 1) * P],
                            qT16[:, k * NB:(k + 1) * NB],
                            start=True, stop=True,
                        )
                        e = est_pool.tile([P, NB], bf16, tag="est")
                        est[(k, mt)] = e
                        nc.scalar.activation(
                            e[:], ps[:], AF.Exp, scale=1.0 / SCALER
                        )

                # --- attention @ w for block k-1 (software-pipelined)
                if k >= 1:
                    mtile = mtp.tile([P, NB], f32, tag="mtp")
                    for mt in range(MT):
                        nc.tensor.matmul(
                            mtile[:],
                            w_sb[:, mt * H:(mt + 1) * H],
                            est[(k - 1, mt)][:],
                            start=(mt == 0), stop=(mt == MT - 1),
                        )
                    mtiles[k - 1] = mtile

                # --- rowsums for block k: two levels of pairwise adds on the
                # Vector engine, then an 8-chunk ones matmul over partitions.
                if k < NT:
                    t2 = []
                    for j in range(MT // 4):
                        t1a = sb_small.tile([P, NB], bf16, tag="t1", bufs=4)
                        nc.vector.tensor_add(
                            t1a[:], est[(k, 4 * j)][:], est[(k, 4 * j + 1)][:]
                        )
                        t1b = sb_small.tile([P, NB], bf16, tag="t1", bufs=4)
                        nc.vector.tensor_add(
                            t1b[:], est[(k, 4 * j + 2)][:], est[(k, 4 * j + 3)][:]
                        )
                        t2j = sb_small.tile([P, NB], bf16, tag="t2", bufs=10)
                        nc.vector.tensor_add(t2j[:], t1a[:], t1b[:])
                        t2.append(t2j)
                    rs = rsp.tile([1, NB], f32, tag="rsp")
                    for j in range(MT // 4):
                        nc.tensor.matmul(
                            rs[:], ones_bf[:], t2[j][:],
                            start=(j == 0), stop=(j == MT // 4 - 1),
                        )
                    rsinv = sb_small.tile([1, NB], f32, tag="rsinv", bufs=2)
                    nc.vector.reciprocal(rsinv[:], rs[:])
                    bck = sb_small.tile([P, NB], f32, tag="bc", bufs=2)
                    nc.gpsimd.partition_broadcast(bck[:], rsinv[:])
                    bc[k] = bck

                # --- drain block k-1: normalize mid, apply V_up, store
                if k >= 1:
                    msc = sb_small.tile([P, NB], f16, tag="msc", bufs=3)
                    nc.vector.tensor_mul(msc[:], mtiles[k - 1][:], bc[k - 1][:])
                    for lt in range(2):
                        op = otp.tile([P, NB], f32, tag="otp")
                        nc.tensor.matmul(
                            op[:], vu16[:, lt * P:(lt + 1) * P], msc[:],
                            start=True, stop=True,
                        )
                        fin = outfin_pool.tile([P, NB], f32, tag="fin")
                        nc.vector.tensor_copy(fin[:], op[:])
                        nc.gpsimd.dma_start(
                            out_ext[lt * P:(lt + 1) * P, (k - 1) * NB:k * NB],
                            fin[:],
                        )

    if not nc.is_finalized():
        nc.finalize()
    return nc


_GRAPH_CACHE = {}


def _get_graph():
    if "nc" not in _GRAPH_CACHE:
        _GRAPH_CACHE["nc"] = _build()
    return _GRAPH_CACHE["nc"]


def run(inputs: dict, trace: bool = False):
    """Run the SPMD kernel on 8 cores. Returns (output, BassKernelResults)."""
    from concourse.bass_utils import run_bass_kernel_spmd

    x = np.asarray(inputs["x"], dtype=np.float32)
    Q = np.asarray(inputs["Q"], dtype=np.float32)[0]
    K = np.asarray(inputs["K"], dtype=np.float32)[0]
    Vd = np.asarray(inputs["V_down"], dtype=np.float32)[0]
    Vu = np.asarray(inputs["V_up"], dtype=np.float32)[0]

    wq = np.ascontiguousarray(Q).astype(np.float16)
    wk = np.ascontiguousarray(K).astype(np.float16)
    vd = np.ascontiguousarray(Vd).astype(np.float16)
    vu = np.ascontiguousarray(Vu).astype(np.float16)

    in_maps = []
    for b in range(B):
        in_maps.append({
            "xT": np.ascontiguousarray(x[b].T).astype(np.float16),
            "Wq": wq,
            "Wk": wk,
            "Vd": vd,
            "Vu": vu,
        })

    nc = _get_graph()
    res = run_bass_kernel_spmd(nc, in_maps, core_ids=list(range(B)), trace=trace)
    # device output is [L, N] per core; un-transpose during the gather
    out = np.stack([np.asarray(res.results[i]["out"]).T for i in range(B)])
    return np.ascontiguousarray(out, dtype=np.float32), res


def kernel(**inputs) -> np.ndarray:
    out, _ = run(inputs, trace=False)
    return out


# revision 17
# speedup vs baseline: 1.3965x; 1.0046x over previous
"""Trainium2 Bass kernel for nn_Attention_27358941675773.

Reference computation (per batch b):
    q = x @ Q              [N, H]
    k = x @ K              [N, H]
    V = V_down @ V_up      [L, L]
    v = x @ V              [N, L]
    S = q @ k.T / 256      [N, N]
    out = softmax(S) @ v   [N, L]

Sharding: pure data-parallel over batch B=8 across the 8 NeuronCores
(one batch element per core); small params replicated. No collectives.

Per-core kernel strategy (N=4096, L=256, H=128):
  - Inputs are shipped as fp16 (x transposed to [L, N]); all matmuls run
    at full PE rate (1 cyc/row). qT [H,N] and kT [H,N] are computed
    directly in transposed layout so scores can be built as S_T[m, n]
    (keys on partitions) without any transposes.
  - The value path is factored through the rank-H bottleneck:
        out = softmax(S) @ x @ V_down @ V_up
    so the O(N^2) matmul contracts into H=128 columns (w = x @ V_down),
    and V_up is applied after the softmax-normalization - halving the
    PE work of the attention*value product.
  - exp(S_T/256) runs on the Scalar engine straight out of PSUM, written
    as bf16 (scores reach ~±70; exp stays in fp32/bf16 range, so no
    max-subtraction pass is needed).
  - softmax denominator rowsum[n] = sum_m exp(S_T[m,n]): two levels of
    pairwise adds on the Vector engine, then an 8-chunk ones-vector
    matmul accumulated in PSUM (partition-axis reduction).
  - normalization: 1/rowsum is partition-broadcast on GpSimd and applied
    to mid^T = w^T-weighted numerator with one Vector multiply; the
    output is stored TRANSPOSED [L, N] in DRAM and un-transposed on the
    host during the gather.
  - The attention*w matmul of block k-1 is software-pipelined against
    the QK/exp of block k so the Scalar engine's exp stream stays hidden.
"""

import os
import sys

import numpy as np

for _p in ("/opt/trn_rl_repo",):
    if _p not in sys.path and os.path.isdir(_p):
        sys.path.insert(0, _p)

B, N, L, H = 8, 4096, 256, 128
SCALER = 256.0
NB = 512            # query-block (free dim of score tiles)
NT = N // NB        # 8 query blocks
MT = N // 128       # 32 key tiles of 128
P = 128


def _build():
    import concourse.bass as bass
    import concourse.tile as tile
    from concourse import bacc, mybir
    from contextlib import ExitStack

    f32 = mybir.dt.float32
    f16 = mybir.dt.float16
    bf16 = mybir.dt.bfloat16
    AF = mybir.ActivationFunctionType

    nc = bacc.Bacc(
        "TRN2", target_bir_lowering=False, debug=False, num_devices=B
    )

    xT_ext = nc.declare_dram_parameter("xT", [L, N], f16, isOutput=False)
    wq_ext = nc.declare_dram_parameter("Wq", [L, H], f16, isOutput=False)
    wk_ext = nc.declare_dram_parameter("Wk", [L, H], f16, isOutput=False)
    vd_ext = nc.declare_dram_parameter("Vd", [L, H], f16, isOutput=False)
    vu_ext = nc.declare_dram_parameter("Vu", [H, L], f16, isOutput=False)
    # output stored transposed [L, N]; host un-transposes at gather
    out_ext = nc.declare_dram_parameter("out", [L, N], f32, isOutput=True)

    with tile.TileContext(nc) as tc, ExitStack() as ctx:
        persist = ctx.enter_context(tc.tile_pool(name="persist", bufs=1))

        ones_bf = persist.tile([P, 1], bf16)
        nc.gpsimd.memset(ones_bf[:], 1.0)

        qw16 = persist.tile([P, 2 * H], f16)    # Q   [l_chunk][l_in, h]
        kw16 = persist.tile([P, 2 * H], f16)
        vd16 = persist.tile([P, 2 * H], f16)    # V_down [l_chunk][l_in, h]
        vu16 = persist.tile([P, L], f16)        # V_up   [h, l]
        xt16 = [
            [
                persist.tile(
                    [P, 1024], f16, tag=f"xt{c}_{s}", name=f"xt16_{c}_{s}"
                )
                for s in range(4)
            ]
            for c in range(2)
        ]
        qT16 = persist.tile([P, N], f16)        # q.T       [h, n]
        kT16 = persist.tile([P, N], f16)        # k.T       [h, m]
        w_sb = persist.tile([P, MT * H], bf16)  # x@V_down  [m_tile][m_in, h]

        # ---------------- phase A: direct fp16 loads ----------------
        for c in range(2):
            nc.sync.dma_start(qw16[:, c * H:(c + 1) * H], wq_ext[c * P:(c + 1) * P, :])
            nc.sync.dma_start(kw16[:, c * H:(c + 1) * H], wk_ext[c * P:(c + 1) * P, :])
            nc.sync.dma_start(vd16[:, c * H:(c + 1) * H], vd_ext[c * P:(c + 1) * P, :])
        nc.sync.dma_start(vu16[:], vu_ext[:, :])
        # interleave the two l_chunks so both chunks of an n-range land early;
        # split each chunk across two DMA queues to halve time-to-first-data
        for s in range(4):
            for c in range(2):
                for h2 in range(2):
                    nc.sync.dma_start(
                        xt16[c][s][:, h2 * 512:(h2 + 1) * 512],
                        xT_ext[
                            c * P:(c + 1) * P,
                            s * 1024 + h2 * 512: s * 1024 + (h2 + 1) * 512,
                        ],
                    )

        # ---------------- phase B: qT, kT, w ----------------
        with tc.tile_pool(name="pp", bufs=3, space="PSUM") as pp:
            # qT[h, n] / kT[h, m]
            for w16, dst in ((kw16, kT16), (qw16, qT16)):
                for f in range(N // NB):
                    ps = pp.tile([P, NB], f32, tag="pp")
                    for c in range(2):
                        nc.tensor.matmul(
                            ps[:],
                            w16[:, c * H:(c + 1) * H],
                            xt16[c][f // 2][:, (f % 2) * NB:(f % 2 + 1) * NB],
                            start=(c == 0), stop=(c == 1),
                        )
                    nc.vector.tensor_copy(dst[:, f * NB:(f + 1) * NB], ps[:])

            # w[m, h] = x @ V_down : lhsT = xT chunk [l', m_tile]
            for mt in range(MT):
                ps = pp.tile([P, H], f32, tag="ppw")
                for c in range(2):
                    nc.tensor.matmul(
                        ps[:],
                        xt16[c][mt // 8][:, (mt % 8) * P:(mt % 8 + 1) * P],
                        vd16[:, c * H:(c + 1) * H],
                        start=(c == 0), stop=(c == 1),
                    )
                nc.vector.tensor_copy(w_sb[:, mt * H:(mt + 1) * H], ps[:])

        # ---------------- phase C: attention main loop ----------------
        with (
            tc.tile_pool(name="est", bufs=2 * MT) as est_pool,
            tc.tile_pool(name="sb_small", bufs=4) as sb_small,
            tc.tile_pool(name="outfin", bufs=4) as outfin_pool,
            tc.tile_pool(name="stp", bufs=3, space="PSUM") as stp,
            tc.tile_pool(name="mtp", bufs=2, space="PSUM") as mtp,
            tc.tile_pool(name="otp", bufs=2, space="PSUM") as otp,
            tc.tile_pool(name="rsp", bufs=1, space="PSUM") as rsp,
        ):
            est = {}      # (k, mt) -> bf16 [128, NB] exp score tiles
            mtiles = {}   # k -> psum numerator mid^T [h, n] tile
            bc = {}       # k -> [128, NB] f32 broadcast 1/rowsum

            for k in range(NT + 1):
                # --- QK + exp for block k, interleaved at matmul granularity
                # with attention@w for block k-1. The per-engine instruction
                # order is static, so alternating keeps PE busy on the k-1
                # product whenever a QK matmul would stall on an exp-fed
                # PSUM slot.
                mtile = None
                if k >= 1:
                    mtile = mtp.tile([P, NB], f32, tag="mtp")
                for mt in range(MT):
                    if k < NT:
                        ps = stp.tile([P, NB], f32, tag="stp")
                        nc.tensor.matmul(
                            ps[:],
                            kT16[:, mt * P:(mt + 1) * P],
                            qT16[:, k * NB:(k + 1) * NB],
                            start=True, stop=True,
                        )
                        e = est_pool.tile([P, NB], bf16, tag="est")
                        est[(k, mt)] = e
                        nc.scalar.activation(
                            e[:], ps[:], AF.Exp, scale=1.0 / SCALER
                        )
                    if k >= 1:
                        nc.tensor.matmul(
                            mtile[:],
                            w_sb[:, mt * H:(mt + 1) * H],
                            est[(k - 1, mt)][:],
                            start=(mt == 0), stop=(mt == MT - 1),
                        )
                if k >= 1:
                    mtiles[k - 1] = mtile

                # --- rowsums for block k: two levels of pairwise adds on the
                # Vector engine, then an 8-chunk ones matmul over partitions.
                if k < NT:
                    t2 = []
                    for j in range(MT // 4):
                        t1a = sb_small.tile([P, NB], bf16, tag="t1", bufs=4)
                        nc.vector.tensor_add(
                            t1a[:], est[(k, 4 * j)][:], est[(k, 4 * j + 1)][:]
                        )
                        t1b = sb_small.tile([P, NB], bf16, tag="t1", bufs=4)
                        nc.vector.tensor_add(
                            t1b[:], est[(k, 4 * j + 2)][:], est[(k, 4 * j + 3)][:]
                        )
                        t2j = sb_small.tile([P, NB], bf16, tag="t2", bufs=10)
                        nc.vector.tensor_add(t2j[:], t1a[:], t1b[:])
                        t2.append(t2j)
                    rs = rsp.tile([1, NB], f32, tag="rsp")
                    for j in range(MT // 4):
                        nc.tensor.matmul(
                            rs[:], ones_bf[:], t2[j][:],
                            start=(j == 0), stop=(j == MT // 4 - 1),
                        )
                    rsinv = sb_small.tile([1, NB], f32, tag="rsinv", bufs=2)
                    nc.vector.reciprocal(rsinv[:], rs[:])
                    bck = sb_small.tile([P, NB], f32, tag="bc", bufs=2)
                    nc.gpsimd.partition_broadcast(bck[:], rsinv[:])
                    bc[k] = bck

                # --- drain block k-1: normalize mid, apply V_up, store
                if k >= 1:
                    msc = sb_small.tile([P, NB], f16, tag="msc", bufs=3)
                    nc.vector.tensor_mul(msc[:], mtiles[k - 1][:], bc[k - 1][:])
                    for lt in range(2):
                        op = otp.tile([P, NB], f32, tag="otp")
                        nc.tensor.matmul(
                            op[:], vu16[:, lt * P:(lt + 1) * P], msc[:],
                            start=True, stop=True,
                        )
                        fin = outfin_pool.tile([P, NB], f32, tag="fin")
                        nc.vector.tensor_copy(fin[:], op[:])
                        nc.gpsimd.dma_start(
                            out_ext[lt * P:(lt + 1) * P, (k - 1) * NB:k * NB],
                            fin[:],
                        )

    if not nc.is_finalized():
        nc.finalize()
    return nc


_GRAPH_CACHE = {}


def _get_graph():
    if "nc" not in _GRAPH_CACHE:
        _GRAPH_CACHE["nc"] = _build()
    return _GRAPH_CACHE["nc"]


def run(inputs: dict, trace: bool = False):
    """Run the SPMD kernel on 8 cores. Returns (output, BassKernelResults)."""
    from concourse.bass_utils import run_bass_kernel_spmd

    x = np.asarray(inputs["x"], dtype=np.float32)
    Q = np.asarray(inputs["Q"], dtype=np.float32)[0]
    K = np.asarray(inputs["K"], dtype=np.float32)[0]
    Vd = np.asarray(inputs["V_down"], dtype=np.float32)[0]
    Vu = np.asarray(inputs["V_up"], dtype=np.float32)[0]

    wq = np.ascontiguousarray(Q).astype(np.float16)
    wk = np.ascontiguousarray(K).astype(np.float16)
    vd = np.ascontiguousarray(Vd).astype(np.float16)
    vu = np.ascontiguousarray(Vu).astype(np.float16)

    in_maps = []
    for b in range(B):
        in_maps.append({
            "xT": np.ascontiguousarray(x[b].T).astype(np.float16),
            "Wq": wq,
            "Wk": wk,
            "Vd": vd,
            "Vu": vu,
        })

    nc = _get_graph()
    res = run_bass_kernel_spmd(nc, in_maps, core_ids=list(range(B)), trace=trace)
    # device output is [L, N] per core; un-transpose during the gather
    out = np.stack([np.asarray(res.results[i]["out"]).T for i in range(B)])
    return np.ascontiguousarray(out, dtype=np.float32), res


def kernel(**inputs) -> np.ndarray:
    out, _ = run(inputs, trace=False)
    return out


# revision 20
# speedup vs baseline: 1.5894x; 1.1382x over previous
"""Trainium2 Bass kernel for nn_Attention_27358941675773.

Reference computation (per batch b):
    q = x @ Q              [N, H]
    k = x @ K              [N, H]
    V = V_down @ V_up      [L, L]
    v = x @ V              [N, L]
    S = q @ k.T / 256      [N, N]
    out = softmax(S) @ v   [N, L]

Sharding: pure data-parallel over batch B=8 across the 8 NeuronCores
(one batch element per core); small params replicated. No collectives.

Per-core kernel strategy (N=4096, L=256, H=128):
  - Inputs are shipped as fp16 (x transposed to [L, N]); all matmuls run
    at full PE rate (1 cyc/row). qT [H,N] and kT [H,N] are computed
    directly in transposed layout so scores can be built as S_T[m, n]
    (keys on partitions) without any transposes.
  - The value path is factored through the rank-H bottleneck:
        out = softmax(S) @ x @ V_down @ V_up
    so the O(N^2) matmul contracts into H=128 columns (w = x @ V_down),
    and V_up is applied after the softmax-normalization - halving the
    PE work of the attention*value product.
  - exp(S_T/256) runs on the Scalar engine straight out of PSUM, written
    as bf16 (scores reach ~±70; exp stays in fp32/bf16 range, so no
    max-subtraction pass is needed).
  - softmax denominator rowsum[n] = sum_m exp(S_T[m,n]): two levels of
    pairwise adds on the Vector engine, then an 8-chunk ones-vector
    matmul accumulated in PSUM (partition-axis reduction).
  - normalization: 1/rowsum is partition-broadcast on GpSimd and applied
    to mid^T = w^T-weighted numerator with one Vector multiply; the
    output is stored TRANSPOSED [L, N] in DRAM and un-transposed on the
    host during the gather.
  - The attention*w matmul of block k-1 is software-pipelined against
    the QK/exp of block k so the Scalar engine's exp stream stays hidden.
"""

import os
import sys

import numpy as np

for _p in ("/opt/trn_rl_repo",):
    if _p not in sys.path and os.path.isdir(_p):
        sys.path.insert(0, _p)

B, N, L, H = 8, 4096, 256, 128
SCALER = 256.0
NB = 512            # query-block (free dim of score tiles)
NT = N // NB        # 8 query blocks
MT = N // 128       # 32 key tiles of 128
P = 128


def _build():
    import concourse.bass as bass
    import concourse.tile as tile
    from concourse import bacc, mybir
    from contextlib import ExitStack

    f32 = mybir.dt.float32
    f16 = mybir.dt.float16
    bf16 = mybir.dt.bfloat16
    AF = mybir.ActivationFunctionType

    nc = bacc.Bacc(
        "TRN2", target_bir_lowering=False, debug=False, num_devices=B
    )

    xT_ext = nc.declare_dram_parameter("xT", [L, N], f16, isOutput=False)
    wq_ext = nc.declare_dram_parameter("Wq", [L, H], f16, isOutput=False)
    wk_ext = nc.declare_dram_parameter("Wk", [L, H], f16, isOutput=False)
    vd_ext = nc.declare_dram_parameter("Vd", [L, H], f16, isOutput=False)
    vu_ext = nc.declare_dram_parameter("Vu", [H, L], f16, isOutput=False)
    # output stored transposed [L, N]; host un-transposes at gather
    out_ext = nc.declare_dram_parameter("out", [L, N], f32, isOutput=True)

    with tile.TileContext(nc) as tc, ExitStack() as ctx:
        persist = ctx.enter_context(tc.tile_pool(name="persist", bufs=1))

        ones_bf = persist.tile([P, 1], bf16)
        nc.gpsimd.memset(ones_bf[:], 1.0)
        # touch Exp right away so the ~2.7us ACT table load overlaps the
        # input DMAs instead of delaying the first real exp
        dum = persist.tile([1, 2], f32)
        nc.gpsimd.memset(dum[:], 0.0)
        nc.scalar.activation(dum[:, 1:2], dum[:, 0:1], AF.Exp)

        qw16 = persist.tile([P, 2 * H], f16)    # Q   [l_chunk][l_in, h]
        kw16 = persist.tile([P, 2 * H], f16)
        vd16 = persist.tile([P, 2 * H], f16)    # V_down [l_chunk][l_in, h]
        vu16 = persist.tile([P, L], f16)        # V_up   [h, l]
        xt16 = [
            [
                persist.tile(
                    [P, 1024], f16, tag=f"xt{c}_{s}", name=f"xt16_{c}_{s}"
                )
                for s in range(4)
            ]
            for c in range(2)
        ]
        qT16 = persist.tile([P, N], f16)        # q.T       [h, n]
        kT16 = persist.tile([P, N], f16)        # k.T       [h, m]
        w_sb = persist.tile([P, MT * H], bf16)  # x@V_down  [m_tile][m_in, h]

        # ---------------- phase A: direct fp16 loads ----------------
        for c in range(2):
            nc.sync.dma_start(qw16[:, c * H:(c + 1) * H], wq_ext[c * P:(c + 1) * P, :])
            nc.sync.dma_start(kw16[:, c * H:(c + 1) * H], wk_ext[c * P:(c + 1) * P, :])
            nc.sync.dma_start(vd16[:, c * H:(c + 1) * H], vd_ext[c * P:(c + 1) * P, :])
        nc.sync.dma_start(vu16[:], vu_ext[:, :])
        # interleave the two l_chunks so both chunks of an n-range land early;
        # split each chunk across two DMA queues to halve time-to-first-data
        for s in range(4):
            for c in range(2):
                for h2 in range(2):
                    nc.sync.dma_start(
                        xt16[c][s][:, h2 * 512:(h2 + 1) * 512],
                        xT_ext[
                            c * P:(c + 1) * P,
                            s * 1024 + h2 * 512: s * 1024 + (h2 + 1) * 512,
                        ],
                    )

        # ------------- phases B+C: projections fused with attention -------
        with (
            tc.tile_pool(name="est", bufs=2 * MT) as est_pool,
            tc.tile_pool(name="sb_small", bufs=4) as sb_small,
            tc.tile_pool(name="outfin", bufs=4) as outfin_pool,
            tc.tile_pool(name="stp", bufs=3, space="PSUM") as stp,
            tc.tile_pool(name="mtp", bufs=2, space="PSUM") as mtp,
            tc.tile_pool(name="otp", bufs=2, space="PSUM") as otp,
            tc.tile_pool(name="rsp", bufs=1, space="PSUM") as rsp,
        ):
            est = {}      # (k, mt) -> bf16 [128, NB] exp score tiles
            mtiles = {}   # k -> psum numerator mid^T [h, n] tile
            bc = {}       # k -> [128, NB] f32 broadcast 1/rowsum

            def proj_qkT(w16, dst, f):
                ps = stp.tile([P, NB], f32, tag="stp")
                for c in range(2):
                    nc.tensor.matmul(
                        ps[:],
                        w16[:, c * H:(c + 1) * H],
                        xt16[c][f // 2][:, (f % 2) * NB:(f % 2 + 1) * NB],
                        start=(c == 0), stop=(c == 1),
                    )
                nc.vector.tensor_copy(dst[:, f * NB:(f + 1) * NB], ps[:])

            def proj_w(mt):
                ps = stp.tile([P, NB], f32, tag="stp")
                for c in range(2):
                    nc.tensor.matmul(
                        ps[:, :H],
                        xt16[c][mt // 8][:, (mt % 8) * P:(mt % 8 + 1) * P],
                        vd16[:, c * H:(c + 1) * H],
                        start=(c == 0), stop=(c == 1),
                    )
                nc.vector.tensor_copy(w_sb[:, mt * H:(mt + 1) * H], ps[:, :H])

            def qk_exp(k, mt):
                ps = stp.tile([P, NB], f32, tag="stp")
                nc.tensor.matmul(
                    ps[:],
                    kT16[:, mt * P:(mt + 1) * P],
                    qT16[:, k * NB:(k + 1) * NB],
                    start=True, stop=True,
                )
                e = est_pool.tile([P, NB], bf16, tag="est")
                est[(k, mt)] = e
                nc.scalar.activation(e[:], ps[:], AF.Exp, scale=1.0 / SCALER)

            def rowsums(k):
                # two levels of pairwise adds on the Vector engine, then an
                # 8-chunk ones matmul over the partition axis
                t2 = []
                for j in range(MT // 4):
                    t1a = sb_small.tile([P, NB], bf16, tag="t1", bufs=4)
                    nc.vector.tensor_add(
                        t1a[:], est[(k, 4 * j)][:], est[(k, 4 * j + 1)][:]
                    )
                    t1b = sb_small.tile([P, NB], bf16, tag="t1", bufs=4)
                    nc.vector.tensor_add(
                        t1b[:], est[(k, 4 * j + 2)][:], est[(k, 4 * j + 3)][:]
                    )
                    t2j = sb_small.tile([P, NB], bf16, tag="t2", bufs=10)
                    nc.vector.tensor_add(t2j[:], t1a[:], t1b[:])
                    t2.append(t2j)
                rs = rsp.tile([1, NB], f32, tag="rsp")
                for j in range(MT // 4):
                    nc.tensor.matmul(
                        rs[:], ones_bf[:], t2[j][:],
                        start=(j == 0), stop=(j == MT // 4 - 1),
                    )
                rsinv = sb_small.tile([1, NB], f32, tag="rsinv", bufs=2)
                nc.vector.reciprocal_approx_fast(rsinv[:], rs[:])
                bck = sb_small.tile([P, NB], f32, tag="bc", bufs=2)
                nc.gpsimd.partition_broadcast(bck[:], rsinv[:])
                bc[k] = bck

            def drain(k):
                # normalize mid, apply V_up, store transposed
                msc = sb_small.tile([P, NB], f16, tag="msc", bufs=3)
                nc.vector.tensor_mul(msc[:], mtiles[k][:], bc[k][:])
                for lt in range(2):
                    op = otp.tile([P, NB], f32, tag="otp")
                    nc.tensor.matmul(
                        op[:], vu16[:, lt * P:(lt + 1) * P], msc[:],
                        start=True, stop=True,
                    )
                    fin = outfin_pool.tile([P, NB], f32, tag="fin")
                    nc.vector.tensor_copy(fin[:], op[:])
                    nc.gpsimd.dma_start(
                        out_ext[lt * P:(lt + 1) * P, k * NB:(k + 1) * NB],
                        fin[:],
                    )

            # --- prologue: start the exp stream as early as possible.
            # qT block 0 and kT blocks are interleaved with QK(0) so the
            # Scalar engine gets work within a few us of the xT DMA landing.
            proj_qkT(qw16, qT16, 0)
            for fb in range(8):
                proj_qkT(kw16, kT16, fb)
                for mt in range(4 * fb, 4 * fb + 4):
                    qk_exp(0, mt)
            # remaining projections run on PE under the exp(0) stream
            for f in range(1, 8):
                proj_qkT(qw16, qT16, f)
            for mt in range(MT):
                proj_w(mt)
            rowsums(0)

            # --- steady state: QK(k) interleaved with attention@w(k-1)
            for k in range(1, NT + 1):
                mtile = mtp.tile([P, NB], f32, tag="mtp", name=f"mid_{k}")
                for mt in range(MT):
                    if k < NT:
                        qk_exp(k, mt)
                    nc.tensor.matmul(
                        mtile[:],
                        w_sb[:, mt * H:(mt + 1) * H],
                        est[(k - 1, mt)][:],
                        start=(mt == 0), stop=(mt == MT - 1),
                    )
                mtiles[k - 1] = mtile
                if k < NT:
                    rowsums(k)
                drain(k - 1)

    if not nc.is_finalized():
        nc.finalize()
    return nc


_GRAPH_CACHE = {}


def _get_graph():
    if "nc" not in _GRAPH_CACHE:
        _GRAPH_CACHE["nc"] = _build()
    return _GRAPH_CACHE["nc"]


def run(inputs: dict, trace: bool = False):
    """Run the SPMD kernel on 8 cores. Returns (output, BassKernelResults)."""
    from concourse.bass_utils import run_bass_kernel_spmd

    x = np.asarray(inputs["x"], dtype=np.float32)
    Q = np.asarray(inputs["Q"], dtype=np.float32)[0]
    K = np.asarray(inputs["K"], dtype=np.float32)[0]
    Vd = np.asarray(inputs["V_down"], dtype=np.float32)[0]
    Vu = np.asarray(inputs["V_up"], dtype=np.float32)[0]

    wq = np.ascontiguousarray(Q).astype(np.float16)
    wk = np.ascontiguousarray(K).astype(np.float16)
    vd = np.ascontiguousarray(Vd).astype(np.float16)
    vu = np.ascontiguousarray(Vu).astype(np.float16)

    in_maps = []
    for b in range(B):
        in_maps.append({
            "xT": np.ascontiguousarray(x[b].T).astype(np.float16),
            "Wq": wq,
            "Wk": wk,
            "Vd": vd,
            "Vu": vu,
        })

    nc = _get_graph()
    res = run_bass_kernel_spmd(nc, in_maps, core_ids=list(range(B)), trace=trace)
    # device output is [L, N] per core; un-transpose during the gather
    out = np.stack([np.asarray(res.results[i]["out"]).T for i in range(B)])
    return np.ascontiguousarray(out, dtype=np.float32), res


def kernel(**inputs) -> np.ndarray:
    out, _ = run(inputs, trace=False)
    return out
